# revision 7
# baseline (speedup 1.0000x reference)
"""Cross-attention Trainium2 kernel (8 NeuronCores, batch-data-parallel).

Computes, per batch element b:
    q = x[b] @ Wq            [S, DK]
    k = y[b] @ Wk            [S, DK]
    v = y[b] @ Wv            [S, E]
    p = exp((q @ k.T) / sqrt(E))        (no max-subtraction: logits ~ N(0, .25))
    out[b] = (p @ v) / rowsum(p) + x[b]

All matmuls run in fp8e4 DoubleRow mode (K=256 per matmul, 2x bf16 rate).
Weights are pre-scaled by 16 on-chip so their values sit in fp8's normal
range; the 16*16 factor on scores folds into the exp scale and the 16 on
v folds into the rowsum (the ones column holds 16.0).

Data movement (per core, BL=2 batches):
  - SWDGE D2D cast fp32 -> fp8 into a DRAM bounce ([S, C] fp8), streamed
    in 512-row quarters so compute starts ~15us into the kernel.
  - The bounce is bitcast to bf16 pairs [S, C/2] and xbar DMA-transposed
    into SBUF: tiles xT[t4] = [128 chan-pairs, S]; partition p of tile t4
    holds channels (256*t4 + 2p, +1) interleaved along the free dim.
    These serve directly as DoubleRow *moving* operands ([128, 2, N],
    strides (1, 2)).
  - DoubleRow *stationary* operands must be pair-blocked (LDWEIGHTS
    rejects a stride-1 pair dim), so yT is additionally deinterleaved on
    DVE into yT_blk [128, 2, S] for the V-projection stationary.
  - Weights load as [128, 2, M] fp32 (rows 256*t4+2p+j) and cast to fp8
    with scale=16 on ScalarE (before any exp traffic).

Pipeline (per batch; waves of 512 query columns):
  per quarter qi: qT chunk qi, kT chunk qi, v tiles 4qi..4qi+3, and
  wave-0 scores for those 4 key tiles (so exp has lead time).
  per wave w: AV per 128-query tile with wave w+1's scores interleaved
  into the PE stream; epilogue = DVE (psum * 1/rowsum) + x -> out.
Engine split: PE matmuls; ScalarE exp (+ initial weight casts); DVE all
psum drains, deinterleave, reciprocal, epilogue.
"""

import math

import numpy as np

# Full-problem constants (hardcoded per the harness contract).
B_FULL = 16
N_CORES = 8
S_Q = 2048
S_KV = 2048
C_DIM = 1024  # input feature dim (contraction of the projections)
DK = 256  # q/k head dim
E_DIM = 1024  # v / output dim
P = 128
QS = 512  # stream quarter / wave size
WSCALE = 16.0  # fp8 range pre-scale applied to all weights


class CFG:
    def __init__(self, bl, sq, skv, c, dk, e):
        assert sq % QS == 0 and skv % QS == 0 and c % 256 == 0 and dk == 256
        self.bl = bl  # batches per core
        self.sq = sq
        self.skv = skv
        self.c = c
        self.dk = dk
        self.e = e
        # exp( (q.k) / sqrt(E) ) with both q and k carrying WSCALE
        self.scale = 1.0 / (math.sqrt(e) * WSCALE * WSCALE)


def emit_cross_attention(tc, outs, ins, cfg):
    """Emit the kernel into TileContext `tc`.

    ins = x, y, Wq, Wk, Wv ; outs = out.
    x/y/out: [bl, sq|skv, c|e] fp32. Weights: [c, dk|e] fp32.
    """
    import concourse.mybir as mybir
    from concourse.mybir import ActivationFunctionType as AF
    from concourse.mybir import AluOpType as ALU
    from concourse.mybir import MatmulPerfMode
    from concourse.tile_rust import add_dep_helper

    nc = tc.nc
    bf16 = mybir.dt.bfloat16
    fp8 = mybir.dt.float8e4
    f32 = mybir.dt.float32
    DR = MatmulPerfMode.DoubleRow

    x, y, Wq, Wk, Wv = ins["x"], ins["y"], ins["Wq"], ins["Wk"], ins["Wv"]
    out = outs["out"]

    nt4 = cfg.c // 256  # channel pair-tiles (256 channels each)
    nt = cfg.skv // P  # key tiles
    nkp = nt // 2  # key pair-tiles
    nd = cfg.dk // P  # dk tiles (2)
    nec = cfg.e // 512  # e chunks
    nqx = cfg.sq // QS  # x stream quarters == waves
    nqy = cfg.skv // QS  # y stream quarters
    tq = QS // P  # key tiles per y quarter (4)
    mh_w = QS // P  # query tiles per wave (4)

    # DRAM bounce buffers for the fp8 copies of x and y.
    xb = nc.dram_tensor("xb8", [cfg.bl, cfg.sq, cfg.c], fp8).ap()
    yb = nc.dram_tensor("yb8", [cfg.bl, cfg.skv, cfg.c], fp8).ap()
    xb16 = xb.bitcast(bf16)  # [bl, sq, c/2]
    yb16 = yb.bitcast(bf16)

    pool = tc.alloc_tile_pool(name="main", bufs=1)
    ps_mm = tc.alloc_tile_pool(name="ps_mm", bufs=3, space="PSUM")
    ps_av = tc.alloc_tile_pool(name="ps_av", bufs=2, space="PSUM")
    ps_sm = tc.alloc_tile_pool(name="ps_sm", bufs=1, space="PSUM")

    # ---- weights: [128, 2, M] fp32 staging -> fp8 * WSCALE ---------------
    def load_weight(w_dram, wdim, t4, name):
        tag = "wstage_v" if wdim > 256 else "wstage_s"
        stage = pool.tile([P, 2, wdim], f32, tag=tag, bufs=2,
                          name=f"ws{name}{t4}")
        src = w_dram[256 * t4:256 * (t4 + 1), :].rearrange(
            "(p j) m -> p j m", j=2)
        nc.scalar.dma_start(out=stage[:], in_=src)
        w8 = pool.tile([P, 2, wdim], fp8, tag=f"w8{name}{t4}",
                       name=f"w8{name}{t4}")
        nc.scalar.activation(w8[:], stage[:], AF.Copy, scale=WSCALE)
        return w8

    wk8 = [load_weight(Wk, cfg.dk, t4, "k") for t4 in range(nt4)]
    wq8 = [load_weight(Wq, cfg.dk, t4, "q") for t4 in range(nt4)]

    ones16 = pool.tile([P, 2, 1], fp8, tag="ones", name="ones")
    nc.gpsimd.memset(ones16[:], WSCALE)

    # ---- activation stream: casts + pair transposes + deinterleave ------
    # SWDGE casts on the gpsimd queue; xbar transposes on the sync ring
    # (Tile serializes each transpose group against all in-flight DMAs,
    # so casts are paced to alternate with transpose windows).
    st = {"last_tg": None}

    def int_view(t):
        # [128, S, 2] fp8 pair-interleaved view of a bf16 transpose tile
        return t[:].bitcast(fp8).rearrange("p (s j) -> p s j", j=2)

    def pace(waiter, dependee):
        if waiter is not None and dependee is not None:
            add_dep_helper(waiter.ins, dependee.ins, sync=True,
                           reason="pace dma windows")

    tiles = {}
    for b in range(cfg.bl):
        for which, n in (("x", cfg.sq), ("y", cfg.skv)):
            tiles[(b, which)] = [
                pool.tile([P, n], bf16, tag=f"{which}T", bufs=2 * nt4,
                          name=f"{which}T{b}_{t4}")
                for t4 in range(nt4)
            ]
        tiles[(b, "yblk")] = [
            pool.tile([P, 2, cfg.skv], fp8, tag="yblk", bufs=2 * nt4,
                      name=f"yblk{b}_{t4}")
            for t4 in range(nt4)
        ]

    def stream_quarter(b, which, qi, extra_dep=None):
        src = y if which == "y" else x
        dstb = yb if which == "y" else xb
        dst16 = yb16 if which == "y" else xb16
        ro = qi * QS
        c = nc.gpsimd.dma_start(out=dstb[b][ro:ro + QS, :],
                                in_=src[b][ro:ro + QS, :])
        pace(c, st["last_tg"])
        pace(c, extra_dep)
        tg = None
        for t4 in range(nt4):
            tg = nc.sync.dma_start(
                out=tiles[(b, which)][t4][:, ro:ro + QS],
                in_=dst16[b][ro:ro + QS, t4 * P:(t4 + 1) * P],
                transpose=True,
            )
        st["last_tg"] = tg

    def deint_quarter(b, qi):
        ro = qi * QS
        for t4 in range(nt4):
            nc.vector.tensor_copy(
                tiles[(b, "yblk")][t4][:, :, ro:ro + QS],
                int_view(tiles[(b, "y")][t4])[:, ro:ro + QS, :]
                .transpose([0, 2, 1]),
            )

    # b0 stream: alternate x/y quarters; wv weight stages after x-q0 so
    # their DMAs overlap the casts on the scalar ring.
    stream_quarter(0, "x", 0)
    wv8 = None
    for qi in range(max(nqx, nqy)):
        if qi == 0:
            wv8 = [load_weight(Wv, cfg.e, t4, "v") for t4 in range(nt4)]
        else:
            if qi < nqx:
                stream_quarter(0, "x", qi)
        if qi < nqy:
            stream_quarter(0, "y", qi)
    # Later batches' streams follow immediately (the pacing chain
    # throttles them).  Their deinterleaves are emitted inside the
    # previous batch's wave loop so the in-order DVE queue reaches them
    # without blocking earlier drains.
    for b in range(1, cfg.bl):
        for qi in range(max(nqx, nqy)):
            if qi < nqx:
                stream_quarter(b, "x", qi)
            if qi < nqy:
                stream_quarter(b, "y", qi)

    # ---- compute ---------------------------------------------------------
    def emit_proj_chunk(b, w8s, which, ci, dst):
        # dst[:, md, ci*QS:...] = (x|y)[chunk ci] @ W  (contraction over c)
        xT = tiles[(b, which)]
        for md in range(nd):
            ps = ps_mm.tile([P, QS], f32, tag="mm", name=f"ps_{which}")
            for t4 in range(nt4):
                mov = int_view(xT[t4])[:, ci * QS:(ci + 1) * QS, :] \
                    .transpose([0, 2, 1])
                nc.tensor.matmul(ps[:], w8s[t4][:, :, md * P:(md + 1) * P],
                                 mov, start=(t4 == 0), stop=(t4 == nt4 - 1),
                                 perf_mode=DR)
            nc.vector.tensor_copy(dst[:, md, ci * QS:(ci + 1) * QS], ps[:])

    def emit_v_tile(b, t, v8):
        yblk = tiles[(b, "yblk")]
        ps_v = ps_av.tile([P, cfg.e], f32, tag="av", name="ps_v")
        for t4 in range(nt4):
            stat = yblk[t4][:, :, t * P:(t + 1) * P]
            for ec in range(nec):
                nc.tensor.matmul(ps_v[:, 512 * ec:512 * (ec + 1)],
                                 stat, wv8[t4][:, :, 512 * ec:512 * (ec + 1)],
                                 start=(t4 == 0), stop=(t4 == nt4 - 1),
                                 perf_mode=DR)
        nc.vector.tensor_copy(v8[:, t, :], ps_v[:])

    def emit_score(kT8, qT8, wo, t, pT_w):
        # one key-tile's scores for wave at query offset wo, plus exp
        ps = ps_mm.tile([P, QS], f32, tag="mm", name="ps_s")
        nc.tensor.matmul(ps[:], kT8[:, :, t * P:(t + 1) * P],
                         qT8[:, :, wo:wo + QS], start=True, stop=True,
                         perf_mode=DR)
        nc.scalar.activation(pT_w[t // 2][:, t % 2, :], ps[:], AF.Exp,
                             scale=cfg.scale)

    def make_pT():
        return [
            pool.tile([P, 2, QS], fp8, tag="pT", bufs=2 * nkp,
                      name=f"pT{kp}")
            for kp in range(nkp)
        ]

    for b in range(cfg.bl):
        kT8 = pool.tile([P, nd, cfg.skv], fp8, tag="kT", bufs=2, name="kT")
        qT8 = pool.tile([P, nd, cfg.sq], fp8, tag="qT", bufs=2, name="qT")
        v8 = pool.tile([P, nt, cfg.e], fp8, tag="v8", bufs=1, name="v8")

        pT_cur = make_pT()
        # quarter-granular projections + wave-0 scores.  Deinterleave for
        # this batch's y: inline for b0; for later batches all but the
        # last quarter were emitted during the previous batch's waves.
        for qi in range(max(nqx, nqy)):
            if qi < nqx:
                emit_proj_chunk(b, wq8, "x", qi, qT8)
            if qi < nqy:
                emit_proj_chunk(b, wk8, "y", qi, kT8)
                if b == 0 or qi == nqy - 1:
                    deint_quarter(b, qi)
                for t in range(tq * qi, tq * (qi + 1)):
                    emit_v_tile(b, t, v8)
                for t in range(tq * qi, tq * (qi + 1)):
                    emit_score(kT8, qT8, 0, t, pT_cur)

        # waves
        for w in range(nqx):
            wo = w * QS
            pT_next = make_pT() if w + 1 < nqx else None
            for mh in range(mh_w):
                sm = wo + mh * P
                ps_e = ps_av.tile([P, cfg.e], f32, tag="av", name="ps_e")
                ps_sum = ps_sm.tile([P, 1], f32, tag="sum", name="ps_sum")
                for kp in range(nkp):
                    stat = pT_cur[kp][:, :, mh * P:(mh + 1) * P]
                    for ec in range(nec):
                        nc.tensor.matmul(
                            ps_e[:, 512 * ec:512 * (ec + 1)],
                            stat, v8[:, 2 * kp:2 * kp + 2,
                                     512 * ec:512 * (ec + 1)],
                            start=(kp == 0), stop=(kp == nkp - 1),
                            perf_mode=DR)
                    nc.tensor.matmul(ps_sum[:], stat, ones16[:],
                                     start=(kp == 0), stop=(kp == nkp - 1),
                                     perf_mode=DR)
                # interleave next wave's scores into the PE stream
                if pT_next is not None:
                    npm = nt // mh_w
                    for t in range(mh * npm, (mh + 1) * npm):
                        emit_score(kT8, qT8, wo + QS, t, pT_next)

                recip = pool.tile([P, 1], f32, tag="recip", bufs=4,
                                  name="recip")
                nc.vector.reciprocal(recip[:], ps_sum[:])
                xres = pool.tile([P, cfg.e], f32, tag="xres", bufs=2,
                                 name="xres")
                nc.scalar.dma_start(out=xres[:], in_=x[b][sm:sm + P, :])
                out_t = pool.tile([P, cfg.e], f32, tag="out_t", bufs=3,
                                  name="out_t")
                nc.vector.scalar_tensor_tensor(
                    out_t[:], ps_e[:], recip[:], xres[:], ALU.mult, ALU.add)
                so = nc.scalar.dma_start(out=out[b][sm:sm + P, :],
                                         in_=out_t[:])
                if w == 0 and mh == 0:
                    st[f"out0_{b}"] = so
            pT_cur = pT_next
            # next batch's deinterleave, one quarter per wave: DVE reaches
            # these between this batch's epilogues; the y stream has
            # landed by then.  (Last quarter is inline in b+1's loop.)
            if b + 1 < cfg.bl and 0 <= w - 1 < nqy - 1:
                deint_quarter(b + 1, w - 1)

    ps_sm.release()
    ps_av.release()
    ps_mm.release()
    pool.release()


def make_tile_kernel(cfg):
    """Adapter with the (tc, outs, ins) signature used by run_kernel/test.py."""

    def k(tc, outs, ins):
        emit_cross_attention(tc, outs, ins, cfg)

    return k


def _build(cfg):
    import concourse.bacc as bacc
    import concourse.mybir as mybir
    import concourse.tile as tile

    f32 = mybir.dt.float32
    nc = bacc.Bacc(
        "TRN2",
        target_bir_lowering=False,
        debug=False,
        enable_asserts=False,
        num_devices=N_CORES,
    )
    ins = {
        "x": nc.dram_tensor("x", [cfg.bl, cfg.sq, cfg.c], f32, kind="ExternalInput").ap(),
        "y": nc.dram_tensor("y", [cfg.bl, cfg.skv, cfg.c], f32, kind="ExternalInput").ap(),
        "Wq": nc.dram_tensor("Wq", [cfg.c, cfg.dk], f32, kind="ExternalInput").ap(),
        "Wk": nc.dram_tensor("Wk", [cfg.c, cfg.dk], f32, kind="ExternalInput").ap(),
        "Wv": nc.dram_tensor("Wv", [cfg.c, cfg.e], f32, kind="ExternalInput").ap(),
    }
    outs = {
        "out": nc.dram_tensor("out", [cfg.bl, cfg.sq, cfg.e], f32, kind="ExternalOutput").ap()
    }
    with tile.TileContext(nc) as tc:
        emit_cross_attention(tc, outs, ins, cfg)
    nc.compile()
    return nc


_CACHED = {}


def run_on_cores(x, y, Wq, Wk, Wv, trace=False):
    from concourse import bass_utils

    cfg = CFG(B_FULL // N_CORES, S_Q, S_KV, C_DIM, DK, E_DIM)
    key = "full"
    if key not in _CACHED:
        _CACHED[key] = _build(cfg)
    nc = _CACHED[key]

    bl = cfg.bl
    in_maps = [
        {
            "x": np.ascontiguousarray(x[i * bl : (i + 1) * bl]),
            "y": np.ascontiguousarray(y[i * bl : (i + 1) * bl]),
            "Wq": Wq,
            "Wk": Wk,
            "Wv": Wv,
        }
        for i in range(N_CORES)
    ]
    res = bass_utils.run_bass_kernel_spmd(
        nc, in_maps, core_ids=list(range(N_CORES)), trace=trace
    )
    out = np.concatenate([r["out"] for r in res.results], axis=0)
    return out, res


def kernel(x, y, Wq, Wk, Wv):
    x = np.asarray(x, dtype=np.float32)
    y = np.asarray(y, dtype=np.float32)
    Wq = np.asarray(Wq, dtype=np.float32)
    Wk = np.asarray(Wk, dtype=np.float32)
    Wv = np.asarray(Wv, dtype=np.float32)
    out, _ = run_on_cores(x, y, Wq, Wk, Wv, trace=False)
    return out


# revision 9
# speedup vs baseline: 1.2213x; 1.2213x over previous
"""Cross-attention Trainium2 kernel (8 NeuronCores, batch-data-parallel).

Computes, per batch element b:
    q = x[b] @ Wq            [S, DK]
    k = y[b] @ Wk            [S, DK]
    v = y[b] @ Wv            [S, E]
    p = exp((q @ k.T) / sqrt(E))        (no max-subtraction: logits ~ N(0, .25))
    out[b] = (p @ v) / rowsum(p) + x[b]

All matmuls run in fp8e4 DoubleRow mode (K=256 per matmul, 2x bf16 rate).
Weights are pre-scaled by 16 on-chip so their values sit in fp8's normal
range; the 16*16 factor on scores folds into the exp scale and the 16 on
v folds into the rowsum (the ones column holds 16.0).  The output store
is bf16 (upcast to fp32 on the host); the residual add itself is fp32.

Data movement (per core, BL=2 batches):
  - SWDGE D2D cast fp32 -> fp8 into a DRAM bounce ([S, C] fp8), streamed
    in 1024-row halves, order x-h1, y-h1, x-h2, y-h2.
  - The bounce is bitcast to bf16 pairs [S, C/2] and xbar DMA-transposed
    into SBUF: tiles xT[t4] = [128 chan-pairs, S]; partition p of tile t4
    holds channels (256*t4 + 2p, +1) interleaved along the free dim.
    These serve directly as DoubleRow *moving* operands ([128, 2, N],
    strides (1, 2)).  Transpose windows serialize against all in-flight
    DMAs, so windows are kept few (4 per batch) and casts are paced to
    alternate with them; weight stages all precede the stream.
  - DoubleRow *stationary* operands must be pair-blocked (LDWEIGHTS
    rejects a stride-1 pair dim), so yT is additionally deinterleaved on
    DVE into yT_blk [128, 2, S] for the V-projection stationary.

Pipeline (per batch; waves of 512 query columns):
  per half h: qT chunks 2h,2h+1; kT ditto; deint; then v tiles with
  wave-0 scores interleaved (exp lead time).  Per wave w: AV per
  128-query tile with wave w+1's scores interleaved into the PE stream;
  epilogue = DVE (psum * 1/rowsum) + x -> bf16 out.
Engine split: PE matmuls (plus a short warmup burst to hold the HAM
clock at 2.4 GHz through the DMA ramp); ScalarE exp + initial weight
casts; DVE psum drains, deinterleave, reciprocal, epilogue.
"""

import math

import numpy as np

# Full-problem constants (hardcoded per the harness contract).
B_FULL = 16
N_CORES = 8
S_Q = 2048
S_KV = 2048
C_DIM = 1024  # input feature dim (contraction of the projections)
DK = 256  # q/k head dim
E_DIM = 1024  # v / output dim
P = 128
QS = 512  # wave size (query cols)
HS = 1024  # stream half size (rows)
WSCALE = 16.0  # fp8 range pre-scale applied to all weights
N_WARMUP = 24  # PE warmup matmuls


class CFG:
    def __init__(self, bl, sq, skv, c, dk, e):
        assert sq % HS == 0 and skv % HS == 0 and c % 256 == 0 and dk == 256
        self.bl = bl  # batches per core
        self.sq = sq
        self.skv = skv
        self.c = c
        self.dk = dk
        self.e = e
        # exp( (q.k) / sqrt(E) ) with both q and k carrying WSCALE
        self.scale = 1.0 / (math.sqrt(e) * WSCALE * WSCALE)


def emit_cross_attention(tc, outs, ins, cfg):
    """Emit the kernel into TileContext `tc`.

    ins = x, y, Wq, Wk, Wv ; outs = out.
    x/y: [bl, sq|skv, c] fp32. Weights: [c, dk|e] fp32. out: bf16.
    """
    import concourse.mybir as mybir
    from concourse.mybir import ActivationFunctionType as AF
    from concourse.mybir import AluOpType as ALU
    from concourse.mybir import MatmulPerfMode
    from concourse.tile_rust import add_dep_helper

    nc = tc.nc
    bf16 = mybir.dt.bfloat16
    fp8 = mybir.dt.float8e4
    f32 = mybir.dt.float32
    DR = MatmulPerfMode.DoubleRow

    x, y, Wq, Wk, Wv = ins["x"], ins["y"], ins["Wq"], ins["Wk"], ins["Wv"]
    out = outs["out"]

    nt4 = cfg.c // 256  # channel pair-tiles (256 channels each)
    nt = cfg.skv // P  # key tiles
    nkp = nt // 2  # key pair-tiles
    nd = cfg.dk // P  # dk tiles (2)
    nec = cfg.e // 512  # e chunks
    nw = cfg.sq // QS  # waves
    nhx = cfg.sq // HS  # x stream halves
    nhy = cfg.skv // HS  # y stream halves
    th = HS // P  # key tiles per y half (8)
    mh_w = QS // P  # query tiles per wave (4)

    # DRAM bounce buffers for the fp8 copies of x and y.
    xb = nc.dram_tensor("xb8", [cfg.bl, cfg.sq, cfg.c], fp8).ap()
    yb = nc.dram_tensor("yb8", [cfg.bl, cfg.skv, cfg.c], fp8).ap()
    xb16 = xb.bitcast(bf16)  # [bl, sq, c/2]
    yb16 = yb.bitcast(bf16)

    pool = tc.alloc_tile_pool(name="main", bufs=1)
    ps_mm = tc.alloc_tile_pool(name="ps_mm", bufs=3, space="PSUM")
    ps_av = tc.alloc_tile_pool(name="ps_av", bufs=2, space="PSUM")
    ps_sm = tc.alloc_tile_pool(name="ps_sm", bufs=1, space="PSUM")

    # ---- PE warmup: keep the HAM clock gate open through the DMA ramp ---
    wu = pool.tile([P, QS], fp8, tag="warm", name="warm")
    nc.gpsimd.memset(wu[:], 1.0)
    wu3 = wu[:].rearrange("p (j n) -> p j n", j=2)
    ps_wu = ps_mm.tile([P, QS // 2], f32, tag="mm", name="ps_wu")
    for _ in range(N_WARMUP):
        nc.tensor.matmul(ps_wu[:], wu3[:, :, :P], wu3, start=True, stop=True,
                         perf_mode=DR)
    nc.vector.tensor_copy(wu[:].bitcast(f32)[:, :64], ps_wu[:, :64])

    # ---- weights: [128, 2, M] fp32 staging -> fp8 * WSCALE, all upfront -
    def load_weight(w_dram, wdim, t4, name):
        tag = "wstage_v" if wdim > 256 else "wstage_s"
        stage = pool.tile([P, 2, wdim], f32, tag=tag, bufs=2,
                          name=f"ws{name}{t4}")
        src = w_dram[256 * t4:256 * (t4 + 1), :].rearrange(
            "(p j) m -> p j m", j=2)
        nc.scalar.dma_start(out=stage[:], in_=src)
        w8 = pool.tile([P, 2, wdim], fp8, tag=f"w8{name}{t4}",
                       name=f"w8{name}{t4}")
        nc.scalar.activation(w8[:], stage[:], AF.Copy, scale=WSCALE)
        return w8

    wk8 = [load_weight(Wk, cfg.dk, t4, "k") for t4 in range(nt4)]
    wq8 = [load_weight(Wq, cfg.dk, t4, "q") for t4 in range(nt4)]
    wv8 = [load_weight(Wv, cfg.e, t4, "v") for t4 in range(nt4)]

    ones16 = pool.tile([P, 2, 1], fp8, tag="ones", name="ones")
    nc.gpsimd.memset(ones16[:], WSCALE)

    # ---- activation stream: casts + pair transposes ---------------------
    st = {"last_tg": None}

    def int_view(t):
        # [128, S, 2] fp8 pair-interleaved view of a bf16 transpose tile
        return t[:].bitcast(fp8).rearrange("p (s j) -> p s j", j=2)

    def pace(waiter, dependee):
        if waiter is not None and dependee is not None:
            add_dep_helper(waiter.ins, dependee.ins, sync=True,
                           reason="pace dma windows")

    tiles = {}
    for b in range(cfg.bl):
        for which, n in (("x", cfg.sq), ("y", cfg.skv)):
            tiles[(b, which)] = [
                pool.tile([P, n], bf16, tag=f"{which}T", bufs=2 * nt4,
                          name=f"{which}T{b}_{t4}")
                for t4 in range(nt4)
            ]
        tiles[(b, "yblk")] = [
            pool.tile([P, 2, cfg.skv], fp8, tag="yblk", bufs=2 * nt4,
                      name=f"yblk{b}_{t4}")
            for t4 in range(nt4)
        ]

    def stream_half(b, which, h):
        src = y if which == "y" else x
        dstb = yb if which == "y" else xb
        dst16 = yb16 if which == "y" else xb16
        ro = h * HS
        c = nc.gpsimd.dma_start(out=dstb[b][ro:ro + HS, :],
                                in_=src[b][ro:ro + HS, :])
        pace(c, st["last_tg"])
        tg = None
        for t4 in range(nt4):
            tg = nc.sync.dma_start(
                out=tiles[(b, which)][t4][:, ro:ro + HS],
                in_=dst16[b][ro:ro + HS, t4 * P:(t4 + 1) * P],
                transpose=True,
            )
        st["last_tg"] = tg

    def deint_half(b, h):
        ro = h * HS
        for t4 in range(nt4):
            nc.vector.tensor_copy(
                tiles[(b, "yblk")][t4][:, :, ro:ro + HS],
                int_view(tiles[(b, "y")][t4])[:, ro:ro + HS, :]
                .transpose([0, 2, 1]),
            )

    for b in range(cfg.bl):
        for h in range(max(nhx, nhy)):
            if h < nhx:
                stream_half(b, "x", h)
            if h < nhy:
                stream_half(b, "y", h)

    # ---- compute ---------------------------------------------------------
    def emit_proj_chunk(b, w8s, which, ci, dst):
        # dst[:, md, ci*QS:...] = (x|y)[chunk ci] @ W  (contraction over c)
        xT = tiles[(b, which)]
        for md in range(nd):
            ps = ps_mm.tile([P, QS], f32, tag="mm", name=f"ps_{which}")
            for t4 in range(nt4):
                mov = int_view(xT[t4])[:, ci * QS:(ci + 1) * QS, :] \
                    .transpose([0, 2, 1])
                nc.tensor.matmul(ps[:], w8s[t4][:, :, md * P:(md + 1) * P],
                                 mov, start=(t4 == 0), stop=(t4 == nt4 - 1),
                                 perf_mode=DR)
            nc.vector.tensor_copy(dst[:, md, ci * QS:(ci + 1) * QS], ps[:])

    def emit_v_tile(b, t, v8):
        yblk = tiles[(b, "yblk")]
        ps_v = ps_av.tile([P, cfg.e], f32, tag="av", name="ps_v")
        for t4 in range(nt4):
            stat = yblk[t4][:, :, t * P:(t + 1) * P]
            for ec in range(nec):
                nc.tensor.matmul(ps_v[:, 512 * ec:512 * (ec + 1)],
                                 stat, wv8[t4][:, :, 512 * ec:512 * (ec + 1)],
                                 start=(t4 == 0), stop=(t4 == nt4 - 1),
                                 perf_mode=DR)
        nc.vector.tensor_copy(v8[:, t, :], ps_v[:])

    def emit_score(kT8, qT8, wo, t, pT_w):
        # one key-tile's scores for wave at query offset wo, plus exp
        ps = ps_mm.tile([P, QS], f32, tag="mm", name="ps_s")
        nc.tensor.matmul(ps[:], kT8[:, :, t * P:(t + 1) * P],
                         qT8[:, :, wo:wo + QS], start=True, stop=True,
                         perf_mode=DR)
        nc.scalar.activation(pT_w[t // 2][:, t % 2, :], ps[:], AF.Exp,
                             scale=cfg.scale)

    def make_pT():
        return [
            pool.tile([P, 2, QS], fp8, tag="pT", bufs=2 * nkp,
                      name=f"pT{kp}")
            for kp in range(nkp)
        ]

    for b in range(cfg.bl):
        kT8 = pool.tile([P, nd, cfg.skv], fp8, tag="kT", bufs=2, name="kT")
        qT8 = pool.tile([P, nd, cfg.sq], fp8, tag="qT", bufs=2, name="qT")
        v8 = pool.tile([P, nt, cfg.e], fp8, tag="v8", bufs=1, name="v8")

        pT_cur = make_pT()
        # half-granular projections, with wave-0 scores interleaved into
        # the v-tile loop so exp gets lead time before AV.
        for h in range(max(nhx, nhy)):
            if h < nhx:
                emit_proj_chunk(b, wq8, "x", 2 * h, qT8)
                emit_proj_chunk(b, wq8, "x", 2 * h + 1, qT8)
            if h < nhy:
                emit_proj_chunk(b, wk8, "y", 2 * h, kT8)
                emit_proj_chunk(b, wk8, "y", 2 * h + 1, kT8)
                if b == 0 or h == nhy - 1:
                    deint_half(b, h)
                for t in range(th * h, th * (h + 1)):
                    emit_v_tile(b, t, v8)
                    emit_score(kT8, qT8, 0, t, pT_cur)

        # waves
        for w in range(nw):
            wo = w * QS
            pT_next = make_pT() if w + 1 < nw else None
            for mh in range(mh_w):
                sm = wo + mh * P
                ps_e = ps_av.tile([P, cfg.e], f32, tag="av", name="ps_e")
                ps_sum = ps_sm.tile([P, 1], f32, tag="sum", name="ps_sum")
                for kp in range(nkp):
                    stat = pT_cur[kp][:, :, mh * P:(mh + 1) * P]
                    for ec in range(nec):
                        nc.tensor.matmul(
                            ps_e[:, 512 * ec:512 * (ec + 1)],
                            stat, v8[:, 2 * kp:2 * kp + 2,
                                     512 * ec:512 * (ec + 1)],
                            start=(kp == 0), stop=(kp == nkp - 1),
                            perf_mode=DR)
                    nc.tensor.matmul(ps_sum[:], stat, ones16[:],
                                     start=(kp == 0), stop=(kp == nkp - 1),
                                     perf_mode=DR)
                # interleave next wave's scores into the PE stream
                if pT_next is not None:
                    npm = nt // mh_w
                    for t in range(mh * npm, (mh + 1) * npm):
                        emit_score(kT8, qT8, wo + QS, t, pT_next)

                recip = pool.tile([P, 1], f32, tag="recip", bufs=4,
                                  name="recip")
                nc.vector.reciprocal(recip[:], ps_sum[:])
                xres = pool.tile([P, cfg.e], f32, tag="xres", bufs=2,
                                 name="xres")
                nc.scalar.dma_start(out=xres[:], in_=x[b][sm:sm + P, :])
                out_t = pool.tile([P, cfg.e], bf16, tag="out_t", bufs=3,
                                  name="out_t")
                nc.vector.scalar_tensor_tensor(
                    out_t[:], ps_e[:], recip[:], xres[:], ALU.mult, ALU.add)
                nc.scalar.dma_start(out=out[b][sm:sm + P, :], in_=out_t[:])
            pT_cur = pT_next
            # next batch's deinterleave, one half per wave: DVE reaches
            # these between this batch's epilogues; the y stream has
            # landed by then.  (Last half is inline in b+1's loop.)
            if b + 1 < cfg.bl and 0 <= w - 1 < nhy - 1:
                deint_half(b + 1, w - 1)

    ps_sm.release()
    ps_av.release()
    ps_mm.release()
    pool.release()


def make_tile_kernel(cfg):
    """Adapter with the (tc, outs, ins) signature used by run_kernel/test.py."""

    def k(tc, outs, ins):
        emit_cross_attention(tc, outs, ins, cfg)

    return k


def _build(cfg):
    import concourse.bacc as bacc
    import concourse.mybir as mybir
    import concourse.tile as tile

    f32 = mybir.dt.float32
    bf16 = mybir.dt.bfloat16
    nc = bacc.Bacc(
        "TRN2",
        target_bir_lowering=False,
        debug=False,
        enable_asserts=False,
        num_devices=N_CORES,
    )
    ins = {
        "x": nc.dram_tensor("x", [cfg.bl, cfg.sq, cfg.c], f32, kind="ExternalInput").ap(),
        "y": nc.dram_tensor("y", [cfg.bl, cfg.skv, cfg.c], f32, kind="ExternalInput").ap(),
        "Wq": nc.dram_tensor("Wq", [cfg.c, cfg.dk], f32, kind="ExternalInput").ap(),
        "Wk": nc.dram_tensor("Wk", [cfg.c, cfg.dk], f32, kind="ExternalInput").ap(),
        "Wv": nc.dram_tensor("Wv", [cfg.c, cfg.e], f32, kind="ExternalInput").ap(),
    }
    outs = {
        "out": nc.dram_tensor("out", [cfg.bl, cfg.sq, cfg.e], bf16, kind="ExternalOutput").ap()
    }
    with tile.TileContext(nc) as tc:
        emit_cross_attention(tc, outs, ins, cfg)
    nc.compile()
    return nc


_CACHED = {}


def run_on_cores(x, y, Wq, Wk, Wv, trace=False):
    from concourse import bass_utils

    cfg = CFG(B_FULL // N_CORES, S_Q, S_KV, C_DIM, DK, E_DIM)
    key = "full"
    if key not in _CACHED:
        _CACHED[key] = _build(cfg)
    nc = _CACHED[key]

    bl = cfg.bl
    in_maps = [
        {
            "x": np.ascontiguousarray(x[i * bl : (i + 1) * bl]),
            "y": np.ascontiguousarray(y[i * bl : (i + 1) * bl]),
            "Wq": Wq,
            "Wk": Wk,
            "Wv": Wv,
        }
        for i in range(N_CORES)
    ]
    res = bass_utils.run_bass_kernel_spmd(
        nc, in_maps, core_ids=list(range(N_CORES)), trace=trace
    )
    out = np.concatenate(
        [np.asarray(r["out"]).astype(np.float32) for r in res.results], axis=0
    )
    return out, res


def kernel(x, y, Wq, Wk, Wv):
    x = np.asarray(x, dtype=np.float32)
    y = np.asarray(y, dtype=np.float32)
    Wq = np.asarray(Wq, dtype=np.float32)
    Wk = np.asarray(Wk, dtype=np.float32)
    Wv = np.asarray(Wv, dtype=np.float32)
    out, _ = run_on_cores(x, y, Wq, Wk, Wv, trace=False)
    return out
